# revision 35
# baseline (speedup 1.0000x reference)
"""Disentangled multi-head attention (DeBERTa-style) on 8 Trainium2 NeuronCores.

Sharding: core c -> batch b = c // 4, head group g = c % 4 (4 of 16 heads).
Each core computes its 4 heads end-to-end (column-parallel QKV projections,
attention, row-parallel slice of the output projection); the host sums the
4 partial outputs per batch in fp32 and adds the bias terms.

Math folds (exact up to bf16 rounding):
  - scores = (q_c.(k_c+k_p) + q_p.k_c) * s as ONE K=128 matmul per tile with
    qcat = [q_c*s ; q_p*s], kcat = [k_c+k_p ; k_c] (scale folded into weights).
  - All four projections (q_c, q_p, k_c, k_p) run at full PE density
    (depth-1024 dense stationaries, two heads packed per 128 columns);
    kcat rows 0:64 are assembled as k_c + k_p on the DVE during the
    PSUM->SBUF move (k_p staged through SBUF by the ACT engine).
  - k-side projection biases are dropped: they add a per-query constant to
    every score row, which softmax cancels exactly.
  - gate: Wg*(1/s) replicated across 128 stationary columns -> the matmul
    with q_c*s yields the partition-broadcast pre-activation directly;
    Sigmoid(+bg) on ACT emits the bf16 gate tile.
  - gate*spatial_bias accumulated into score PSUM via identity matmul.
  - softmax without max-subtraction (scores bounded ~+-8, fp32-exp safe).
  - exp batched 1024-wide across two PSUM banks per ACT instruction.
  - row-sums from a packed ones-column in the token-major ctx matmul, so
    normalization is per-partition reciprocal + tensor_scalar multiply.
  - ctx accumulation runs s0/s1 over groups 0..5 first (two-pass) so the
    last exp's latency hides under earlier ctx matmuls.
  - bq/bpq added per-partition during the ACT PSUM->SBUF copies; bv and bo
    folded on host (softmax rows sum to 1 when mask is all-True).

DMA choreography: weights -> xkk pair0 -> xkk pair1 -> xv -> xq pair0 ->
sbt ch0, with per-chunk sbt/xq prefetch afterwards; spatial_bias is streamed
per query-chunk ([128,512] tiles) instead of held resident, and the xv region
is reused for it.
"""

import sys

sys.path.insert(0, "/opt/trn_rl_repo")

from contextlib import ExitStack

import numpy as np
import ml_dtypes

import concourse.bass as bass
from concourse import mybir, masks
from concourse.tile import TileContext
from concourse.bass_utils import run_bass_kernel_spmd

BF16 = ml_dtypes.bfloat16

B, L, D = 2, 2048, 1024
H = 16
HK = 64          # head dim
NCORES = 8
HPC = 4          # heads per core
CS = HPC * HK    # channels per core = 256
NJ = L // 128    # 16 key/token blocks
NCH = L // 512   # 4 query chunks
KB_D = D // 128  # 8 contraction blocks for a 1024-deep dim
SCALE = float(1.0 / np.sqrt(HK))

_FP32 = mybir.dt.float32
_BF16 = mybir.dt.bfloat16
_EXP = mybir.ActivationFunctionType.Exp
_SIG = mybir.ActivationFunctionType.Sigmoid
_IDENT = mybir.ActivationFunctionType.Identity
_COPY = mybir.ActivationFunctionType.Copy

# column offsets inside the merged weight wall [1024, 1280]
WC_KC, WC_KP, WC_V, WC_QC, WC_QP = 0, 256, 512, 768, 1024


def _split_multiwaits(nc, skip_opcodes=()):
    """This walrus build encodes at most one sync-wait per TPB instruction.
    Tile attaches several; hoist the extras onto same-engine NoOps placed
    immediately before the instruction (engines are in-order, so semantics
    are preserved)."""
    nsplit = 0
    for fn in nc.m.functions:
        for blk in fn.blocks:
            insts = blk.instructions
            out = []
            for inst in insts:
                si = inst.sync_info
                waits = list(si.on_wait) if si is not None and si.on_wait else []
                if len(waits) > 1 and inst.opcode not in skip_opcodes:
                    si.on_wait = waits[-1:]
                    for i, w in enumerate(waits[:-1]):
                        nop = mybir.InstNoOp(name=f"{inst.name}-w{i}",
                                             ins=[], outs=[])
                        nop.engine = inst.engine
                        nop.sync_info = type(si)(on_wait=[w], on_update=[])
                        out.append(nop)
                    nsplit += 1
                out.append(inst)
            if len(out) != len(insts):
                blk.instructions = out
    return nsplit


def build_nc():
    """Emit the per-core BIR (identical on all 8 cores; data differs)."""
    nc = bass.Bass()

    xq = nc.dram_tensor("xq", [2 * D, L], _BF16, kind="ExternalInput")
    xkk = nc.dram_tensor("xkk", [2 * D, L], _BF16, kind="ExternalInput")
    xv = nc.dram_tensor("xv", [D, L], _BF16, kind="ExternalInput")
    sbt = nc.dram_tensor("sbt", [L, L], _BF16, kind="ExternalInput")
    wall = nc.dram_tensor("wall", [D, 1280], _BF16, kind="ExternalInput")
    wo = nc.dram_tensor("wo", [CS, D], _BF16, kind="ExternalInput")
    smb = nc.dram_tensor("smb", [128, 128], _BF16, kind="ExternalInput")
    smf = nc.dram_tensor("smf", [128, 8], _FP32, kind="ExternalInput")
    outT = nc.dram_tensor("outT", [D, L], _BF16, kind="ExternalOutput")

    with TileContext(nc) as tc, ExitStack() as top:
        pool = lambda **kw: top.enter_context(tc.tile_pool(**kw))

        const_pool = pool(name="const", bufs=1)
        w_pool = pool(name="w", bufs=1)
        x_pool = pool(name="xin", bufs=24)
        qk_pool = pool(name="qkres", bufs=1)
        v_pool = pool(name="vres", bufs=1)
        ksb_pool = pool(name="ksb", bufs=2)
        gb_pool = pool(name="gb", bufs=5)
        tmp_pool = pool(name="tmp", bufs=9)
        e_pool = pool(name="et", bufs=17)
        ctx_pool = pool(name="csb", bufs=4)
        inv_pool = pool(name="inv", bufs=4)
        cta_pool = pool(name="cta", bufs=1)
        oute_pool = pool(name="oute", bufs=2)

        big_pool = pool(name="big", bufs=4, space="PSUM")

        ident = const_pool.tile([128, 128], _BF16, tag="ident", name="ident")
        masks.make_identity(nc, ident[:])

        # ---- weights ----------------------------------------------------
        wall_t = []

        def load_wall(kb):
            t = w_pool.tile([128, 1280], _BF16, tag=f"wall{kb}", name=f"wall{kb}")
            nc.sync.dma_start(t[:], wall[kb * 128:(kb + 1) * 128, :])
            wall_t.append(t)

        smb_t = const_pool.tile([128, 128], _BF16, tag="smb", name="smbt")
        smf_t = const_pool.tile([128, 8], _FP32, tag="smf", name="smft")

        def issue_x(src, pair):
            """16 [128,1024] tiles covering chunks 2*pair, 2*pair+1."""
            csl = slice(pair * 1024, (pair + 1) * 1024)
            ts = []
            for kb in range(KB_D):
                xc = x_pool.tile([128, 1024], _BF16, tag="x", name="xct")
                nc.sync.dma_start(xc[:], src[kb * 128:(kb + 1) * 128, csl])
                xp = x_pool.tile([128, 1024], _BF16, tag="x", name="xpt")
                nc.sync.dma_start(xp[:], src[D + kb * 128:D + (kb + 1) * 128, csl])
                ts.append((xc, xp))
            return ts

        def issue_x_interleaved(src, pair):
            csl = slice(pair * 1024, (pair + 1) * 1024)
            ts = []
            for kb in range(KB_D):
                load_wall(kb)
                xc = x_pool.tile([128, 1024], _BF16, tag="x", name="xct")
                nc.sync.dma_start(xc[:], src[kb * 128:(kb + 1) * 128, csl])
                xp = x_pool.tile([128, 1024], _BF16, tag="x", name="xpt")
                nc.sync.dma_start(xp[:], src[D + kb * 128:D + (kb + 1) * 128, csl])
                ts.append((xc, xp))
            return ts

        for kb in range(KB_D):
            load_wall(kb)
        nc.sync.dma_start(smb_t[:], smb[:, :])
        nc.sync.dma_start(smf_t[:], smf[:, :])
        xk_tiles = [issue_x(xkk, 0)]

        wo_t = []
        for kb in range(2):
            t = w_pool.tile([128, D], _BF16, tag=f"wo{kb}", name=f"wo{kb}")
            nc.sync.dma_start(t[:], wo[kb * 128:(kb + 1) * 128, :])
            wo_t.append(t)

        xk_tiles.append(issue_x(xkk, 1))

        qcat = [qk_pool.tile([128, L], _BF16, tag=f"qcat{h}", name=f"qcat{h}") for h in range(HPC)]
        kcat = [qk_pool.tile([128, L], _BF16, tag=f"kcat{h}", name=f"kcat{h}") for h in range(HPC)]
        vones = [[None] * NJ for _ in range(HPC)]

        # ---- P2-k: k_c / k_p projections, dense, 2 heads packed ---------
        for ch in range(NCH):
            csl = slice(ch * 512, (ch + 1) * 512)
            tsl = slice((ch % 2) * 512, (ch % 2) * 512 + 512)
            tiles = xk_tiles[ch // 2]
            psk = [big_pool.tile([128, 1024], _FP32, tag="big", name="psk")
                   for _ in range(2)]
            for kb in range(KB_D):
                xc, xp = tiles[kb]
                for p in range(2):
                    nc.tensor.matmul(psk[p][:, 0:512],
                                     wall_t[kb][:, WC_KC + p * 128:WC_KC + (p + 1) * 128],
                                     xc[:, tsl], start=(kb == 0), stop=(kb == KB_D - 1))
                    nc.tensor.matmul(psk[p][:, 512:1024],
                                     wall_t[kb][:, WC_KP + p * 128:WC_KP + (p + 1) * 128],
                                     xp[:, tsl], start=(kb == 0), stop=(kb == KB_D - 1))
            for p in range(2):
                ksb = ksb_pool.tile([128, 512], _BF16, tag="ksb", name="ksbt")
                nc.scalar.activation(ksb[:], psk[p][:, 512:1024], _COPY)
                for hh in range(2):
                    h = 2 * p + hh
                    rows = slice(hh * 64, (hh + 1) * 64)
                    nc.vector.tensor_add(kcat[h][0:64, csl],
                                         psk[p][rows, 0:512], ksb[rows, :])
                    nc.vector.tensor_copy(kcat[h][64:128, csl],
                                          psk[p][rows, 0:512])

        # ---- P1: v projection (token-major); xv region reused for sbt ---
        with tc.tile_pool(name="xv_rows", bufs=KB_D) as xvr_pool:
            xv_t = []
            for kb in range(KB_D):
                t = xvr_pool.tile([128, L], _BF16, tag="xvr", name="xvr")
                nc.sync.dma_start(t[:], xv[kb * 128:(kb + 1) * 128, :])
                xv_t.append(t)
            for tb in range(NJ):
                ps = big_pool.tile([128, 1024], _FP32, tag="big", name="psv")
                for kb in range(KB_D):
                    nc.tensor.matmul(
                        ps[:, 0:CS], xv_t[kb][:, tb * 128:(tb + 1) * 128],
                        wall_t[kb][:, WC_V:WC_V + CS],
                        start=(kb == 0), stop=(kb == KB_D - 1))
                for h in range(HPC):
                    vb = v_pool.tile([128, 65], _BF16, tag=f"vb{h}_{tb}", name=f"vb{h}_{tb}")
                    nc.scalar.activation(vb[:, 0:HK], ps[:, h * HK:(h + 1) * HK], _COPY)
                    nc.gpsimd.memset(vb[:, HK:65], 1.0)
                    vones[h][tb] = vb

        # sbt streamed per query-chunk into the region xv vacated
        sbt_pool = pool(name="sbt", bufs=28)
        sbt_c = [None] * NCH

        def issue_sbt(ch):
            ts = []
            for j in range(NJ):
                t = sbt_pool.tile([128, 512], _BF16, tag="sbt", name="sbtt")
                nc.sync.dma_start(t[:], sbt[j * 128:(j + 1) * 128,
                                            ch * 512:(ch + 1) * 512])
                ts.append(t)
            sbt_c[ch] = ts

        xq_tiles = [issue_x(xq, 0), None]
        issue_sbt(0)

        # ---- per chunk: q-projections, gate, attention, output ----------
        cta = [cta_pool.tile([128, L], _BF16, tag=f"cta{k}", name=f"cta{k}") for k in range(2)]
        gbs = [None] * NCH
        tmps = {}

        def mk_tmp(h, ch, g):
            """gate*spatial_bias products for score group g, emitted ahead of
            the PE matmuls that consume them; alternate DVE / Pool (gpsimd)
            so neither engine's queue backs up."""
            for jj in range(2):
                j = 2 * g + jj
                t = tmp_pool.tile([128, 512], _BF16, tag="tmp", name="tmpt")
                nc.vector.tensor_mul(t[:], gbs[ch][h][:], sbt_c[ch][j][:])
                tmps[(h, ch, g, jj)] = t

        def emit_qp2_pair(ch, p):
            """Project q_c/q_p for head pair p of chunk ch, then per-head
            PSUM->SBUF copies + gate sigmoid. Pair 1 is deferred until after
            A(h0) so the boundary ACT burst is halved."""
            csl = slice(ch * 512, (ch + 1) * 512)
            tsl = slice((ch % 2) * 512, (ch % 2) * 512 + 512)
            tiles = xq_tiles[ch // 2]
            psq = big_pool.tile([128, 1024], _FP32, tag="big", name="psq")
            for kb in range(KB_D):
                xc, xp = tiles[kb]
                nc.tensor.matmul(psq[:, 0:512],
                                 wall_t[kb][:, WC_QC + p * 128:WC_QC + (p + 1) * 128],
                                 xc[:, tsl], start=(kb == 0), stop=(kb == KB_D - 1))
                nc.tensor.matmul(psq[:, 512:1024],
                                 wall_t[kb][:, WC_QP + p * 128:WC_QP + (p + 1) * 128],
                                 xp[:, tsl], start=(kb == 0), stop=(kb == KB_D - 1))
            if gbs[ch] is None:
                gbs[ch] = {}
            for hh in range(2):
                h = 2 * p + hh
                rows = slice(hh * 64, (hh + 1) * 64)
                nc.vector.tensor_scalar_add(qcat[h][0:64, csl], psq[rows, 0:512],
                                            smf_t[rows, p:p + 1])
                nc.vector.tensor_scalar_add(qcat[h][64:128, csl], psq[rows, 512:1024],
                                            smf_t[rows, 2 + p:3 + p])
                psg = big_pool.tile([128, 1024], _FP32, tag="big", name="psg")
                nc.tensor.matmul(psg[:, 0:512], smb_t[0:HK, :], qcat[h][0:HK, csl])
                g = gb_pool.tile([128, 512], _BF16, tag="gb", name="gbt")
                nc.scalar.activation(g[:], psg[:, 0:512], _SIG,
                                     bias=smf_t[:, 4 + h:5 + h])
                gbs[ch][h] = g
                if p == 0 and hh == 0:
                    # head 0's gate products go ahead of head 1's copies so
                    # the first identity matmuls are never DVE-gated
                    mk_tmp(0, ch, 0)
                    mk_tmp(0, ch, 1)

        def emit_o(ch):
            csl = slice(ch * 512, (ch + 1) * 512)
            for obp in range(KB_D // 2):
                pso = big_pool.tile([128, 1024], _FP32, tag="big", name="pso")
                for oh in range(2):
                    ob = 2 * obp + oh
                    sl = slice(oh * 512, (oh + 1) * 512)
                    for kb in range(2):
                        nc.tensor.matmul(
                            pso[:, sl], wo_t[kb][:, ob * 128:(ob + 1) * 128],
                            cta[kb][:, csl], start=(kb == 0), stop=(kb == 1))
                    ot = oute_pool.tile([128, 512], _BF16, tag="ot", name="ott")
                    nc.vector.tensor_copy(ot[:], pso[:, sl])
                    nc.sync.dma_start(
                        outT[ob * 128:(ob + 1) * 128, csl], ot[:])

        def emit_scores(h, ch, nxt):
            """Score + gate-bias matmuls and exps for one head/chunk unit."""
            csl = slice(ch * 512, (ch + 1) * 512)
            ets = []
            for g in range(NJ // 2):
                pss = big_pool.tile([128, 1024], _FP32, tag="big", name="pss")
                for jj in range(2):
                    j = 2 * g + jj
                    sl = slice(jj * 512, (jj + 1) * 512)
                    nc.tensor.matmul(pss[:, sl], kcat[h][:, j * 128:(j + 1) * 128],
                                     qcat[h][:, csl], start=True, stop=False)
                    nc.tensor.matmul(pss[:, sl], ident[:],
                                     tmps.pop((h, ch, g, jj))[:],
                                     start=False, stop=True)
                if g + 2 < NJ // 2:
                    mk_tmp(h, ch, g + 2)
                et = e_pool.tile([128, 1024], _BF16, tag="et", name="ett")
                nc.scalar.activation(et[:], pss[:], _EXP)
                ets.append(et)
            if nxt is not None and gbs[nxt[1]] and nxt[0] in gbs[nxt[1]]:
                mk_tmp(nxt[0], nxt[1], 0)
                mk_tmp(nxt[0], nxt[1], 1)
            return ets

        def emit_ctx(h, ch, ets):
            """Deferred ctx phase: runs while the NEXT unit's scores occupy
            ACT, keeping both engines dense (software pipelining)."""
            rows = slice((h % 2) * HK, (h % 2) * HK + HK)

            def ctx_mm(pctx, s, g0, g1, start, stop):
                for g in range(g0, g1):
                    for jj in range(2):
                        j = 2 * g + jj
                        nc.tensor.matmul(
                            pctx[:],
                            ets[g][:, jj * 512 + s * 128: jj * 512 + (s + 1) * 128],
                            vones[h][j][:],
                            start=(start and g == g0 and jj == 0),
                            stop=(stop and g == g1 - 1 and jj == 1))

            def ctx_fin(pctx, s):
                inv = inv_pool.tile([128, 1], _FP32, tag="inv", name="invt")
                nc.vector.reciprocal(inv[:], pctx[:, HK:65])
                csb = ctx_pool.tile([128, HK], _BF16, tag="csb", name="csbt")
                nc.vector.tensor_scalar_mul(csb[:], pctx[:, 0:HK], inv[:])
                # transpose in place into the retired pctx bank (bf16 view)
                ptv = pctx[:, 0:HK].bitcast(_BF16)
                nc.tensor.matmul(ptv[rows, :], csb[:], ident[:],
                                 is_transpose=True)
                col = ch * 512 + s * 128
                nc.vector.tensor_copy(cta[h // 2][rows, col:col + 128],
                                      ptv[rows, :])

            # each big slot hosts two ctx accumulators (one per bank)
            pcA = big_pool.tile([128, 1024], _FP32, tag="big", name="pcA")
            p01 = [pcA[:, 0:65], pcA[:, 512:577]]
            for s in range(2):
                ctx_mm(p01[s], s, 0, 8, True, True)
                ctx_fin(p01[s], s)
            pcB = big_pool.tile([128, 1024], _FP32, tag="big", name="pcB")
            p23 = [pcB[:, 0:65], pcB[:, 512:577]]
            for s in range(2, 4):
                ctx_mm(p23[s - 2], s, 0, 8, True, True)
                ctx_fin(p23[s - 2], s)

        units = [(h, ch) for ch in range(NCH) for h in range(HPC)]
        pend = None
        emit_qp2_pair(0, 0)
        for i, (h, ch) in enumerate(units):
            nxt = units[i + 1] if i + 1 < len(units) else None
            if h == 0:
                # prefetch next chunk's inputs
                if ch == 0:
                    xq_tiles[1] = issue_x(xq, 1)
                if ch < NCH - 1:
                    issue_sbt(ch + 1)
            if h == HPC - 1 and ch + 1 < NCH:
                # next chunk's first projection pair runs inside this unit's
                # window, so its copies/gates never stall the boundary
                emit_qp2_pair(ch + 1, 0)
            ets = emit_scores(h, ch, nxt)
            if pend is not None:
                emit_ctx(*pend)
            pend = (h, ch, ets)
            if h == 0:
                emit_qp2_pair(ch, 1)
            if h == 1 and ch > 0:
                emit_o(ch - 1)
        emit_ctx(*pend)
        emit_o(NCH - 1)

    _split_multiwaits(nc)
    return nc


_NC_CACHE = {}


def _get_nc():
    if "nc" not in _NC_CACHE:
        _NC_CACHE["nc"] = build_nc()
    return _NC_CACHE["nc"]


def _np_reference(k, v, q, mask, spatial_bias, pos_k, pos_q,
                  Wk, bk, Wv, bv, Wq, bq, Wpk, bpk, Wpq, bpq, Wo, bo, Wg, bg):
    """Slow numpy fallback (only if mask is not all-True)."""
    def lin(x, W, b):
        return x @ W.T + b

    def split(x):
        return x.reshape(B, L, H, -1).transpose(0, 2, 1, 3)

    k_c, v_c, q_c = split(lin(k, Wk, bk)), split(lin(v, Wv, bv)), split(lin(q, Wq, bq))
    k_p, q_p = split(lin(pos_k, Wpk, bpk)), split(lin(pos_q, Wpq, bpq))
    scores = (np.einsum("bhqd,bhkd->bhqk", q_c, k_c)
              + np.einsum("bhqd,bhkd->bhqk", q_c, k_p)
              + np.einsum("bhqd,bhkd->bhqk", q_p, k_c)) * SCALE
    gate = 1.0 / (1.0 + np.exp(-(q_c @ Wg.T + bg)))
    scores = scores + gate * spatial_bias
    scores = np.where(mask[:, None, :, :], scores, -np.inf)
    scores = scores - scores.max(-1, keepdims=True)
    e = np.exp(scores)
    attn = e / e.sum(-1, keepdims=True)
    ctx = np.einsum("bhqk,bhkd->bhqd", attn, v_c)
    ctx = ctx.transpose(0, 2, 1, 3).reshape(B, L, D)
    return lin(ctx, Wo, bo).astype(np.float32)


def kernel(k, v, q, mask, spatial_bias, pos_k, pos_q,
           Wk, bk, Wv, bv, Wq, bq, Wpk, bpk, Wpq, bpq, Wo, bo, Wg, bg,
           **_unused):
    f32 = lambda x: np.asarray(x, np.float32)
    k, v, q, pos_k, pos_q = f32(k), f32(v), f32(q), f32(pos_k), f32(pos_q)
    spatial_bias = f32(spatial_bias)
    mask = np.asarray(mask)
    Wk, Wv, Wq, Wpk, Wpq, Wo, Wg = map(f32, (Wk, Wv, Wq, Wpk, Wpq, Wo, Wg))
    bk, bv, bq, bpk, bpq, bo, bg = map(f32, (bk, bv, bq, bpk, bpq, bo, bg))

    if not mask.all():
        return _np_reference(k, v, q, mask, spatial_bias, pos_k, pos_q,
                             Wk, bk, Wv, bv, Wq, bq, Wpk, bpk, Wpq, bpq,
                             Wo, bo, Wg, bg)

    nc = _get_nc()

    def t_bf16(x):  # [L, D] -> [D, L] bf16
        return np.ascontiguousarray(x.T).astype(BF16)

    xq_b = [np.ascontiguousarray(
        np.vstack([q[b].T, pos_q[b].T])).astype(BF16) for b in range(B)]
    xkk_b = [np.ascontiguousarray(
        np.vstack([k[b].T, pos_k[b].T])).astype(BF16) for b in range(B)]
    xv_b = [t_bf16(v[b]) for b in range(B)]
    sbt_b = [np.ascontiguousarray(spatial_bias[b, 0].T).astype(BF16)
             for b in range(B)]

    WqT, WpqT = Wq.T * SCALE, Wpq.T * SCALE
    WkT, WpkT, WvT, WoT = Wk.T, Wpk.T, Wv.T, Wo.T
    smb_a = np.zeros((128, 128), np.float32)
    smb_a[0:HK, :] = np.repeat((Wg[0] * (1.0 / SCALE))[:, None], 128, axis=1)
    in_maps = []
    for c in range(NCORES):
        b, g = c // 4, c % 4
        cs = slice(g * CS, (g + 1) * CS)
        wall_a = np.empty((D, 1280), np.float32)
        wall_a[:, WC_KC:WC_KC + CS] = WkT[:, cs]
        wall_a[:, WC_KP:WC_KP + CS] = WpkT[:, cs]
        wall_a[:, WC_V:WC_V + CS] = WvT[:, cs]
        wall_a[:, WC_QC:WC_QC + CS] = WqT[:, cs]
        wall_a[:, WC_QP:WC_QP + CS] = WpqT[:, cs]
        # smf: col 0/1 = qc bias pair0/1, col 2/3 = qp bias, col 4..7 = bg
        smf_a = np.zeros((128, 8), np.float32)
        for p in range(2):
            for hh in range(2):
                chs = slice(g * CS + (2 * p + hh) * HK,
                            g * CS + (2 * p + hh + 1) * HK)
                smf_a[hh * 64:(hh + 1) * 64, p] = bq[chs] * SCALE
                smf_a[hh * 64:(hh + 1) * 64, 2 + p] = bpq[chs] * SCALE
        smf_a[:, 4:8] = float(bg[0])
        in_maps.append({
            "xq": xq_b[b], "xkk": xkk_b[b], "xv": xv_b[b], "sbt": sbt_b[b],
            "wall": wall_a.astype(BF16),
            "wo": np.ascontiguousarray(WoT[cs, :]).astype(BF16),
            "smb": smb_a.astype(BF16), "smf": smf_a,
        })

    res = run_bass_kernel_spmd(nc, in_maps, core_ids=list(range(NCORES)))

    const_row = (bv @ WoT + bo).astype(np.float32)  # exact bv/bo fold
    out = np.empty((B, L, D), np.float32)
    for b in range(B):
        acc = res.results[b * 4]["outT"].astype(np.float32, copy=True)
        for g in range(1, 4):
            acc += res.results[b * 4 + g]["outT"]
        out[b] = acc.T + const_row
    return out


# revision 36
# speedup vs baseline: 1.0109x; 1.0109x over previous
"""Disentangled multi-head attention (DeBERTa-style) on 8 Trainium2 NeuronCores.

Sharding: core c -> batch b = c // 4, head group g = c % 4 (4 of 16 heads).
Each core computes its 4 heads end-to-end (column-parallel QKV projections,
attention, row-parallel slice of the output projection); the host sums the
4 partial outputs per batch in fp32 and adds the bias terms.

Math folds (exact up to bf16 rounding):
  - scores = (q_c.(k_c+k_p) + q_p.k_c) * s as ONE K=128 matmul per tile with
    qcat = [q_c*s ; q_p*s], kcat = [k_c+k_p ; k_c] (scale folded into weights).
  - All four projections (q_c, q_p, k_c, k_p) run at full PE density
    (depth-1024 dense stationaries, two heads packed per 128 columns);
    kcat rows 0:64 are assembled as k_c + k_p on the DVE during the
    PSUM->SBUF move (k_p staged through SBUF by the ACT engine).
  - k-side projection biases are dropped: they add a per-query constant to
    every score row, which softmax cancels exactly.
  - gate: Wg*(1/s) replicated across 128 stationary columns -> the matmul
    with q_c*s yields the partition-broadcast pre-activation directly;
    Sigmoid(+bg) on ACT emits the bf16 gate tile.
  - gate*spatial_bias accumulated into score PSUM via identity matmul.
  - softmax without max-subtraction (scores bounded ~+-8, fp32-exp safe).
  - exp batched 1024-wide across two PSUM banks per ACT instruction.
  - row-sums from a packed ones-column in the token-major ctx matmul, so
    normalization is per-partition reciprocal + tensor_scalar multiply.
  - ctx accumulation runs s0/s1 over groups 0..5 first (two-pass) so the
    last exp's latency hides under earlier ctx matmuls.
  - bq/bpq added per-partition during the ACT PSUM->SBUF copies; bv and bo
    folded on host (softmax rows sum to 1 when mask is all-True).

DMA choreography: weights -> xkk pair0 -> xkk pair1 -> xv -> xq pair0 ->
sbt ch0, with per-chunk sbt/xq prefetch afterwards; spatial_bias is streamed
per query-chunk ([128,512] tiles) instead of held resident, and the xv region
is reused for it.
"""

import sys

sys.path.insert(0, "/opt/trn_rl_repo")

from contextlib import ExitStack

import numpy as np
import ml_dtypes

import concourse.bass as bass
from concourse import mybir, masks
from concourse.tile import TileContext
from concourse.bass_utils import run_bass_kernel_spmd

BF16 = ml_dtypes.bfloat16

B, L, D = 2, 2048, 1024
H = 16
HK = 64          # head dim
NCORES = 8
HPC = 4          # heads per core
CS = HPC * HK    # channels per core = 256
NJ = L // 128    # 16 key/token blocks
NCH = L // 512   # 4 query chunks
KB_D = D // 128  # 8 contraction blocks for a 1024-deep dim
SCALE = float(1.0 / np.sqrt(HK))

_FP32 = mybir.dt.float32
_BF16 = mybir.dt.bfloat16
_EXP = mybir.ActivationFunctionType.Exp
_SIG = mybir.ActivationFunctionType.Sigmoid
_IDENT = mybir.ActivationFunctionType.Identity
_COPY = mybir.ActivationFunctionType.Copy

# column offsets inside the merged weight wall [1024, 1280]
WC_KC, WC_KP, WC_V, WC_QC, WC_QP = 0, 256, 512, 768, 1024


def _split_multiwaits(nc, skip_opcodes=()):
    """This walrus build encodes at most one sync-wait per TPB instruction.
    Tile attaches several; hoist the extras onto same-engine NoOps placed
    immediately before the instruction (engines are in-order, so semantics
    are preserved)."""
    nsplit = 0
    for fn in nc.m.functions:
        for blk in fn.blocks:
            insts = blk.instructions
            out = []
            for inst in insts:
                si = inst.sync_info
                waits = list(si.on_wait) if si is not None and si.on_wait else []
                if len(waits) > 1 and inst.opcode not in skip_opcodes:
                    si.on_wait = waits[-1:]
                    for i, w in enumerate(waits[:-1]):
                        nop = mybir.InstNoOp(name=f"{inst.name}-w{i}",
                                             ins=[], outs=[])
                        nop.engine = inst.engine
                        nop.sync_info = type(si)(on_wait=[w], on_update=[])
                        out.append(nop)
                    nsplit += 1
                out.append(inst)
            if len(out) != len(insts):
                blk.instructions = out
    return nsplit


def build_nc():
    """Emit the per-core BIR (identical on all 8 cores; data differs)."""
    nc = bass.Bass()

    xq = nc.dram_tensor("xq", [2 * D, L], _BF16, kind="ExternalInput")
    xkk = nc.dram_tensor("xkk", [2 * D, L], _BF16, kind="ExternalInput")
    xv = nc.dram_tensor("xv", [D, L], _BF16, kind="ExternalInput")
    sbt = nc.dram_tensor("sbt", [L, L], _BF16, kind="ExternalInput")
    wall = nc.dram_tensor("wall", [D, 1280], _BF16, kind="ExternalInput")
    wo = nc.dram_tensor("wo", [CS, D], _BF16, kind="ExternalInput")
    smb = nc.dram_tensor("smb", [128, 128], _BF16, kind="ExternalInput")
    smf = nc.dram_tensor("smf", [128, 8], _FP32, kind="ExternalInput")
    outT = nc.dram_tensor("outT", [D, L], _BF16, kind="ExternalOutput")

    with TileContext(nc) as tc, ExitStack() as top:
        pool = lambda **kw: top.enter_context(tc.tile_pool(**kw))

        const_pool = pool(name="const", bufs=1)
        w_pool = pool(name="w", bufs=1)
        x_pool = pool(name="xin", bufs=24)
        qk_pool = pool(name="qkres", bufs=1)
        v_pool = pool(name="vres", bufs=1)
        ksb_pool = pool(name="ksb", bufs=2)
        gb_pool = pool(name="gb", bufs=5)
        tmp_pool = pool(name="tmp", bufs=9)
        e_pool = pool(name="et", bufs=17)
        ctx_pool = pool(name="csb", bufs=4)
        inv_pool = pool(name="inv", bufs=4)
        cta_pool = pool(name="cta", bufs=1)
        oute_pool = pool(name="oute", bufs=2)

        big_pool = pool(name="big", bufs=4, space="PSUM")

        ident = const_pool.tile([128, 128], _BF16, tag="ident", name="ident")
        masks.make_identity(nc, ident[:])

        # ---- weights ----------------------------------------------------
        wall_t = []

        def load_wall(kb):
            t = w_pool.tile([128, 1280], _BF16, tag=f"wall{kb}", name=f"wall{kb}")
            nc.sync.dma_start(t[:], wall[kb * 128:(kb + 1) * 128, :])
            wall_t.append(t)

        smb_t = const_pool.tile([128, 128], _BF16, tag="smb", name="smbt")
        smf_t = const_pool.tile([128, 8], _FP32, tag="smf", name="smft")

        def issue_x(src, pair):
            """16 [128,1024] tiles covering chunks 2*pair, 2*pair+1."""
            csl = slice(pair * 1024, (pair + 1) * 1024)
            ts = []
            for kb in range(KB_D):
                xc = x_pool.tile([128, 1024], _BF16, tag="x", name="xct")
                nc.sync.dma_start(xc[:], src[kb * 128:(kb + 1) * 128, csl])
                xp = x_pool.tile([128, 1024], _BF16, tag="x", name="xpt")
                nc.sync.dma_start(xp[:], src[D + kb * 128:D + (kb + 1) * 128, csl])
                ts.append((xc, xp))
            return ts

        def issue_x_interleaved(src, pair):
            csl = slice(pair * 1024, (pair + 1) * 1024)
            ts = []
            for kb in range(KB_D):
                load_wall(kb)
                xc = x_pool.tile([128, 1024], _BF16, tag="x", name="xct")
                nc.sync.dma_start(xc[:], src[kb * 128:(kb + 1) * 128, csl])
                xp = x_pool.tile([128, 1024], _BF16, tag="x", name="xpt")
                nc.sync.dma_start(xp[:], src[D + kb * 128:D + (kb + 1) * 128, csl])
                ts.append((xc, xp))
            return ts

        for kb in range(KB_D):
            load_wall(kb)
        nc.sync.dma_start(smb_t[:], smb[:, :])
        nc.sync.dma_start(smf_t[:], smf[:, :])
        xk_tiles = [issue_x(xkk, 0)]

        wo_t = []
        for kb in range(2):
            t = w_pool.tile([128, D], _BF16, tag=f"wo{kb}", name=f"wo{kb}")
            nc.sync.dma_start(t[:], wo[kb * 128:(kb + 1) * 128, :])
            wo_t.append(t)

        xk_tiles.append(issue_x(xkk, 1))

        qcat = [qk_pool.tile([128, L], _BF16, tag=f"qcat{h}", name=f"qcat{h}") for h in range(HPC)]
        kcat = [qk_pool.tile([128, L], _BF16, tag=f"kcat{h}", name=f"kcat{h}") for h in range(HPC)]
        vones = [[None] * NJ for _ in range(HPC)]

        # ---- P2-k: k_c / k_p projections, dense, 2 heads packed ---------
        for ch in range(NCH):
            csl = slice(ch * 512, (ch + 1) * 512)
            tsl = slice((ch % 2) * 512, (ch % 2) * 512 + 512)
            tiles = xk_tiles[ch // 2]
            psk = [big_pool.tile([128, 1024], _FP32, tag="big", name="psk")
                   for _ in range(2)]
            for kb in range(KB_D):
                xc, xp = tiles[kb]
                for p in range(2):
                    nc.tensor.matmul(psk[p][:, 0:512],
                                     wall_t[kb][:, WC_KC + p * 128:WC_KC + (p + 1) * 128],
                                     xc[:, tsl], start=(kb == 0), stop=(kb == KB_D - 1))
                    nc.tensor.matmul(psk[p][:, 512:1024],
                                     wall_t[kb][:, WC_KP + p * 128:WC_KP + (p + 1) * 128],
                                     xp[:, tsl], start=(kb == 0), stop=(kb == KB_D - 1))
            for p in range(2):
                ksb = ksb_pool.tile([128, 512], _BF16, tag="ksb", name="ksbt")
                nc.scalar.activation(ksb[:], psk[p][:, 512:1024], _COPY)
                for hh in range(2):
                    h = 2 * p + hh
                    rows = slice(hh * 64, (hh + 1) * 64)
                    nc.vector.tensor_add(kcat[h][0:64, csl],
                                         psk[p][rows, 0:512], ksb[rows, :])
                    nc.vector.tensor_copy(kcat[h][64:128, csl],
                                          psk[p][rows, 0:512])

        # ---- P1: v projection (token-major); xv region reused for sbt ---
        with tc.tile_pool(name="xv_rows", bufs=KB_D) as xvr_pool:
            xv_t = []
            for kb in range(KB_D):
                t = xvr_pool.tile([128, L], _BF16, tag="xvr", name="xvr")
                nc.sync.dma_start(t[:], xv[kb * 128:(kb + 1) * 128, :])
                xv_t.append(t)
            for tb in range(NJ):
                ps = big_pool.tile([128, 1024], _FP32, tag="big", name="psv")
                for kb in range(KB_D):
                    nc.tensor.matmul(
                        ps[:, 0:CS], xv_t[kb][:, tb * 128:(tb + 1) * 128],
                        wall_t[kb][:, WC_V:WC_V + CS],
                        start=(kb == 0), stop=(kb == KB_D - 1))
                for h in range(HPC):
                    vb = v_pool.tile([128, 65], _BF16, tag=f"vb{h}_{tb}", name=f"vb{h}_{tb}")
                    nc.vector.tensor_copy(vb[:, 0:HK], ps[:, h * HK:(h + 1) * HK])
                    nc.gpsimd.memset(vb[:, HK:65], 1.0)
                    vones[h][tb] = vb

        # sbt streamed per query-chunk into the region xv vacated
        sbt_pool = pool(name="sbt", bufs=28)
        sbt_c = [None] * NCH

        def issue_sbt(ch):
            ts = []
            for j in range(NJ):
                t = sbt_pool.tile([128, 512], _BF16, tag="sbt", name="sbtt")
                nc.sync.dma_start(t[:], sbt[j * 128:(j + 1) * 128,
                                            ch * 512:(ch + 1) * 512])
                ts.append(t)
            sbt_c[ch] = ts

        xq_tiles = [issue_x(xq, 0), None]
        issue_sbt(0)

        # ---- per chunk: q-projections, gate, attention, output ----------
        cta = [cta_pool.tile([128, L], _BF16, tag=f"cta{k}", name=f"cta{k}") for k in range(2)]
        gbs = [None] * NCH
        tmps = {}

        def mk_tmp(h, ch, g):
            """gate*spatial_bias products for score group g, emitted ahead of
            the PE matmuls that consume them; alternate DVE / Pool (gpsimd)
            so neither engine's queue backs up."""
            for jj in range(2):
                j = 2 * g + jj
                t = tmp_pool.tile([128, 512], _BF16, tag="tmp", name="tmpt")
                nc.vector.tensor_mul(t[:], gbs[ch][h][:], sbt_c[ch][j][:])
                tmps[(h, ch, g, jj)] = t

        def emit_qp2_pair(ch, p):
            """Project q_c/q_p for head pair p of chunk ch, then per-head
            PSUM->SBUF copies + gate sigmoid. Pair 1 is deferred until after
            A(h0) so the boundary ACT burst is halved."""
            csl = slice(ch * 512, (ch + 1) * 512)
            tsl = slice((ch % 2) * 512, (ch % 2) * 512 + 512)
            tiles = xq_tiles[ch // 2]
            psq = big_pool.tile([128, 1024], _FP32, tag="big", name="psq")
            for kb in range(KB_D):
                xc, xp = tiles[kb]
                nc.tensor.matmul(psq[:, 0:512],
                                 wall_t[kb][:, WC_QC + p * 128:WC_QC + (p + 1) * 128],
                                 xc[:, tsl], start=(kb == 0), stop=(kb == KB_D - 1))
                nc.tensor.matmul(psq[:, 512:1024],
                                 wall_t[kb][:, WC_QP + p * 128:WC_QP + (p + 1) * 128],
                                 xp[:, tsl], start=(kb == 0), stop=(kb == KB_D - 1))
            if gbs[ch] is None:
                gbs[ch] = {}
            for hh in range(2):
                h = 2 * p + hh
                rows = slice(hh * 64, (hh + 1) * 64)
                if hh == 0:
                    nc.vector.tensor_scalar_add(qcat[h][0:64, csl], psq[rows, 0:512],
                                                smf_t[rows, p:p + 1])
                    nc.vector.tensor_scalar_add(qcat[h][64:128, csl], psq[rows, 512:1024],
                                                smf_t[rows, 2 + p:3 + p])
                else:
                    nc.scalar.activation(qcat[h][0:64, csl], psq[rows, 0:512],
                                         _IDENT, bias=smf_t[rows, p:p + 1])
                    nc.scalar.activation(qcat[h][64:128, csl], psq[rows, 512:1024],
                                         _IDENT, bias=smf_t[rows, 2 + p:3 + p])
                psg = big_pool.tile([128, 1024], _FP32, tag="big", name="psg")
                nc.tensor.matmul(psg[:, 0:512], smb_t[0:HK, :], qcat[h][0:HK, csl])
                g = gb_pool.tile([128, 512], _BF16, tag="gb", name="gbt")
                nc.scalar.activation(g[:], psg[:, 0:512], _SIG,
                                     bias=smf_t[:, 4 + h:5 + h])
                gbs[ch][h] = g
                if p == 0 and hh == 0:
                    # head 0's gate products go ahead of head 1's copies so
                    # the first identity matmuls are never DVE-gated
                    mk_tmp(0, ch, 0)
                    mk_tmp(0, ch, 1)

        def emit_o(ch):
            csl = slice(ch * 512, (ch + 1) * 512)
            for obp in range(KB_D // 2):
                pso = big_pool.tile([128, 1024], _FP32, tag="big", name="pso")
                for oh in range(2):
                    ob = 2 * obp + oh
                    sl = slice(oh * 512, (oh + 1) * 512)
                    for kb in range(2):
                        nc.tensor.matmul(
                            pso[:, sl], wo_t[kb][:, ob * 128:(ob + 1) * 128],
                            cta[kb][:, csl], start=(kb == 0), stop=(kb == 1))
                    ot = oute_pool.tile([128, 512], _BF16, tag="ot", name="ott")
                    nc.vector.tensor_copy(ot[:], pso[:, sl])
                    nc.sync.dma_start(
                        outT[ob * 128:(ob + 1) * 128, csl], ot[:])

        def emit_scores(h, ch, nxt):
            """Score + gate-bias matmuls and exps for one head/chunk unit."""
            csl = slice(ch * 512, (ch + 1) * 512)
            ets = []
            for g in range(NJ // 2):
                pss = big_pool.tile([128, 1024], _FP32, tag="big", name="pss")
                for jj in range(2):
                    j = 2 * g + jj
                    sl = slice(jj * 512, (jj + 1) * 512)
                    nc.tensor.matmul(pss[:, sl], kcat[h][:, j * 128:(j + 1) * 128],
                                     qcat[h][:, csl], start=True, stop=False)
                    nc.tensor.matmul(pss[:, sl], ident[:],
                                     tmps.pop((h, ch, g, jj))[:],
                                     start=False, stop=True)
                if g + 2 < NJ // 2:
                    mk_tmp(h, ch, g + 2)
                et = e_pool.tile([128, 1024], _BF16, tag="et", name="ett")
                nc.scalar.activation(et[:], pss[:], _EXP)
                ets.append(et)
            if nxt is not None and gbs[nxt[1]] and nxt[0] in gbs[nxt[1]]:
                mk_tmp(nxt[0], nxt[1], 0)
                mk_tmp(nxt[0], nxt[1], 1)
            return ets

        def emit_ctx(h, ch, ets):
            """Deferred ctx phase: runs while the NEXT unit's scores occupy
            ACT, keeping both engines dense (software pipelining)."""
            rows = slice((h % 2) * HK, (h % 2) * HK + HK)

            def ctx_mm(pctx, s, g0, g1, start, stop):
                for g in range(g0, g1):
                    for jj in range(2):
                        j = 2 * g + jj
                        nc.tensor.matmul(
                            pctx[:],
                            ets[g][:, jj * 512 + s * 128: jj * 512 + (s + 1) * 128],
                            vones[h][j][:],
                            start=(start and g == g0 and jj == 0),
                            stop=(stop and g == g1 - 1 and jj == 1))

            def ctx_fin(pctx, s):
                inv = inv_pool.tile([128, 1], _FP32, tag="inv", name="invt")
                nc.vector.reciprocal(inv[:], pctx[:, HK:65])
                csb = ctx_pool.tile([128, HK], _BF16, tag="csb", name="csbt")
                nc.vector.tensor_scalar_mul(csb[:], pctx[:, 0:HK], inv[:])
                # transpose in place into the retired pctx bank (bf16 view)
                ptv = pctx[:, 0:HK].bitcast(_BF16)
                nc.tensor.matmul(ptv[rows, :], csb[:], ident[:],
                                 is_transpose=True)
                col = ch * 512 + s * 128
                nc.vector.tensor_copy(cta[h // 2][rows, col:col + 128],
                                      ptv[rows, :])

            # each big slot hosts two ctx accumulators (one per bank)
            pcA = big_pool.tile([128, 1024], _FP32, tag="big", name="pcA")
            p01 = [pcA[:, 0:65], pcA[:, 512:577]]
            for s in range(2):
                ctx_mm(p01[s], s, 0, 8, True, True)
                ctx_fin(p01[s], s)
            pcB = big_pool.tile([128, 1024], _FP32, tag="big", name="pcB")
            p23 = [pcB[:, 0:65], pcB[:, 512:577]]
            for s in range(2, 4):
                ctx_mm(p23[s - 2], s, 0, 8, True, True)
                ctx_fin(p23[s - 2], s)

        units = [(h, ch) for ch in range(NCH) for h in range(HPC)]
        pend = None
        emit_qp2_pair(0, 0)
        for i, (h, ch) in enumerate(units):
            nxt = units[i + 1] if i + 1 < len(units) else None
            if h == 0:
                # prefetch next chunk's inputs
                if ch == 0:
                    xq_tiles[1] = issue_x(xq, 1)
                if ch < NCH - 1:
                    issue_sbt(ch + 1)
            if h == HPC - 1 and ch + 1 < NCH:
                # next chunk's first projection pair runs inside this unit's
                # window, so its copies/gates never stall the boundary
                emit_qp2_pair(ch + 1, 0)
            ets = emit_scores(h, ch, nxt)
            if pend is not None:
                emit_ctx(*pend)
            pend = (h, ch, ets)
            if h == 0:
                emit_qp2_pair(ch, 1)
            if h == 1 and ch > 0:
                emit_o(ch - 1)
        emit_ctx(*pend)
        emit_o(NCH - 1)

    _split_multiwaits(nc)
    return nc


_NC_CACHE = {}


def _get_nc():
    if "nc" not in _NC_CACHE:
        _NC_CACHE["nc"] = build_nc()
    return _NC_CACHE["nc"]


def _np_reference(k, v, q, mask, spatial_bias, pos_k, pos_q,
                  Wk, bk, Wv, bv, Wq, bq, Wpk, bpk, Wpq, bpq, Wo, bo, Wg, bg):
    """Slow numpy fallback (only if mask is not all-True)."""
    def lin(x, W, b):
        return x @ W.T + b

    def split(x):
        return x.reshape(B, L, H, -1).transpose(0, 2, 1, 3)

    k_c, v_c, q_c = split(lin(k, Wk, bk)), split(lin(v, Wv, bv)), split(lin(q, Wq, bq))
    k_p, q_p = split(lin(pos_k, Wpk, bpk)), split(lin(pos_q, Wpq, bpq))
    scores = (np.einsum("bhqd,bhkd->bhqk", q_c, k_c)
              + np.einsum("bhqd,bhkd->bhqk", q_c, k_p)
              + np.einsum("bhqd,bhkd->bhqk", q_p, k_c)) * SCALE
    gate = 1.0 / (1.0 + np.exp(-(q_c @ Wg.T + bg)))
    scores = scores + gate * spatial_bias
    scores = np.where(mask[:, None, :, :], scores, -np.inf)
    scores = scores - scores.max(-1, keepdims=True)
    e = np.exp(scores)
    attn = e / e.sum(-1, keepdims=True)
    ctx = np.einsum("bhqk,bhkd->bhqd", attn, v_c)
    ctx = ctx.transpose(0, 2, 1, 3).reshape(B, L, D)
    return lin(ctx, Wo, bo).astype(np.float32)


def kernel(k, v, q, mask, spatial_bias, pos_k, pos_q,
           Wk, bk, Wv, bv, Wq, bq, Wpk, bpk, Wpq, bpq, Wo, bo, Wg, bg,
           **_unused):
    f32 = lambda x: np.asarray(x, np.float32)
    k, v, q, pos_k, pos_q = f32(k), f32(v), f32(q), f32(pos_k), f32(pos_q)
    spatial_bias = f32(spatial_bias)
    mask = np.asarray(mask)
    Wk, Wv, Wq, Wpk, Wpq, Wo, Wg = map(f32, (Wk, Wv, Wq, Wpk, Wpq, Wo, Wg))
    bk, bv, bq, bpk, bpq, bo, bg = map(f32, (bk, bv, bq, bpk, bpq, bo, bg))

    if not mask.all():
        return _np_reference(k, v, q, mask, spatial_bias, pos_k, pos_q,
                             Wk, bk, Wv, bv, Wq, bq, Wpk, bpk, Wpq, bpq,
                             Wo, bo, Wg, bg)

    nc = _get_nc()

    def t_bf16(x):  # [L, D] -> [D, L] bf16
        return np.ascontiguousarray(x.T).astype(BF16)

    xq_b = [np.ascontiguousarray(
        np.vstack([q[b].T, pos_q[b].T])).astype(BF16) for b in range(B)]
    xkk_b = [np.ascontiguousarray(
        np.vstack([k[b].T, pos_k[b].T])).astype(BF16) for b in range(B)]
    xv_b = [t_bf16(v[b]) for b in range(B)]
    sbt_b = [np.ascontiguousarray(spatial_bias[b, 0].T).astype(BF16)
             for b in range(B)]

    WqT, WpqT = Wq.T * SCALE, Wpq.T * SCALE
    WkT, WpkT, WvT, WoT = Wk.T, Wpk.T, Wv.T, Wo.T
    smb_a = np.zeros((128, 128), np.float32)
    smb_a[0:HK, :] = np.repeat((Wg[0] * (1.0 / SCALE))[:, None], 128, axis=1)
    in_maps = []
    for c in range(NCORES):
        b, g = c // 4, c % 4
        cs = slice(g * CS, (g + 1) * CS)
        wall_a = np.empty((D, 1280), np.float32)
        wall_a[:, WC_KC:WC_KC + CS] = WkT[:, cs]
        wall_a[:, WC_KP:WC_KP + CS] = WpkT[:, cs]
        wall_a[:, WC_V:WC_V + CS] = WvT[:, cs]
        wall_a[:, WC_QC:WC_QC + CS] = WqT[:, cs]
        wall_a[:, WC_QP:WC_QP + CS] = WpqT[:, cs]
        # smf: col 0/1 = qc bias pair0/1, col 2/3 = qp bias, col 4..7 = bg
        smf_a = np.zeros((128, 8), np.float32)
        for p in range(2):
            for hh in range(2):
                chs = slice(g * CS + (2 * p + hh) * HK,
                            g * CS + (2 * p + hh + 1) * HK)
                smf_a[hh * 64:(hh + 1) * 64, p] = bq[chs] * SCALE
                smf_a[hh * 64:(hh + 1) * 64, 2 + p] = bpq[chs] * SCALE
        smf_a[:, 4:8] = float(bg[0])
        in_maps.append({
            "xq": xq_b[b], "xkk": xkk_b[b], "xv": xv_b[b], "sbt": sbt_b[b],
            "wall": wall_a.astype(BF16),
            "wo": np.ascontiguousarray(WoT[cs, :]).astype(BF16),
            "smb": smb_a.astype(BF16), "smf": smf_a,
        })

    res = run_bass_kernel_spmd(nc, in_maps, core_ids=list(range(NCORES)))

    const_row = (bv @ WoT + bo).astype(np.float32)  # exact bv/bo fold
    out = np.empty((B, L, D), np.float32)
    for b in range(B):
        acc = res.results[b * 4]["outT"].astype(np.float32, copy=True)
        for g in range(1, 4):
            acc += res.results[b * 4 + g]["outT"]
        out[b] = acc.T + const_row
    return out
